# revision 44
# baseline (speedup 1.0000x reference)
"""ConvVMamba TRN2 Bass kernel (v4).

Sharding: data-parallel over batch. B=8 -> one image per NeuronCore, all
weights replicated, no collectives.

Layout: channels on SBUF partitions (C=96), pixels on the free dim
(L=64*64=4096).

v4 pipeline design (the v2 baseline serialized phases; v3 pipelined but
hit in-order head-of-line blocking). Rules used here:
  - every engine queue is in-order: a stalled op blocks everything behind
    it on that engine, so each engine's emission stream must be laid out
    so successive ops' deps resolve in roughly emission order.
  - conv7/conv3/all 1x1-GEMM passes flow through one 4-deep [128,512]
    psum rotation (cv0-3), keeping the PE stream dense; wide [*,1024]
    psum tiles (st, fa) serve LN stats and fc accumulation.
  - LN rstd via chunked ACT Ln(var+eps) + Exp(-0.5 ln) (DVE reciprocal
    measured 6.4ns/col on HW - unusable).
  - LN var pass adds eps via a 97th input row preset to 96*eps against a
    1/96 ones lhsT; mean pass uses a -1/96 lhsT (bf16 src) or an
    exact-1.0 f32r lhsT with the -1/96 folded into the d-STT (f32 src;
    f32r lhsT values must be bf16-exact or the PE mangles them).
  - fc1/mfc1/ip biases ride a 97th lhsT row against a preset ones-row in
    the normalized-input tiles, so gelu is the only fc1 evac op.
  - ALL weight DMAs use 96/128-partition dram tensors: 97-partition DMA
    loads corrupt subsequent bf16 DMA loads (found empirically) - hence
    bias-row weights are padded to 128 rows.
  - ACT function sets: 6 (Ln+Exp+Copy+Identity), 10 (Gelu), 18 (Silu),
    2 (Sigmoid); ACT ops are chained in emission order with explicit
    table loads between function phases (~9 loads total).

Selective scan (d_state=1, A=-1): dA = sigmoid(-(z+dtb)) in one ACT op,
lnd = Ln(dA) = -delta, du = lnd*u, bso = du*(-B.u) = delta*u*(B.u) via a
rank-1 broadcast matmul, h = tensor_tensor_scan(dA, bso), y = h*(C.u).
Directions 1,3 run w-major via strided views of v4; directions 2,3 run
their scan through reversed APs.
"""

import sys
import numpy as np

sys.path.insert(0, "/opt/trn_rl_repo")

import ml_dtypes  # noqa: E402
import concourse.bass as bass  # noqa: E402
import concourse.bacc as bacc  # noqa: E402
import concourse.mybir as mybir  # noqa: E402
from concourse import tile  # noqa: E402
from concourse.tile import add_dep_helper  # noqa: E402
from concourse.bass_utils import run_bass_kernel_spmd  # noqa: E402

F32 = mybir.dt.float32
F32R = mybir.dt.float32r
BF16 = mybir.dt.bfloat16
AF = mybir.ActivationFunctionType
OP = mybir.AluOpType
bfnp = ml_dtypes.bfloat16

B, C, H, W = 8, 96, 64, 64
L = H * W
R, N, K = 6, 1, 4
EPS = 1e-5
P7, P3 = 70, 66
NCH = 8
CH = 512
NPR = 4
PR = 1024

_CACHE = {}


def _taps(k):
    r = (k - 1) // 2
    return [(dh, dw) for dh in range(k) for dw in range(k)], r


def build_host_tensors(kw):
    f = lambda a: np.asarray(a, np.float32)
    out = {}

    def fold(wname, bname, g, b):
        w = f(kw[wname])
        bb = f(kw[bname])
        return w * f(g)[None, :], bb + w @ f(b)

    fc1w, fc1b = fold("cn_fc1_w", "cn_fc1_b", kw["cn_ln_w"], kw["cn_ln_b"])
    ipw, ipb = fold("ip_w", "ip_b", kw["v_ln1_w"], kw["v_ln1_b"])
    opw, opb = fold("op_w", "op_b", kw["o_ln_w"], kw["o_ln_b"])
    mfc1w, mfc1b = fold("m_fc1_w", "m_fc1_b", kw["v_ln2_w"], kw["v_ln2_b"])
    fc2w, fc2b = f(kw["cn_fc2_w"]), f(kw["cn_fc2_b"])
    mfc2w, mfc2b = f(kw["m_fc2_w"]), f(kw["m_fc2_b"])

    # depthwise conv diagonals, 32x32 blocks: [96, 58*32]
    w7 = f(kw["cn_dw_w"]).reshape(C, 49)
    w3 = f(kw["dw_w"]).reshape(C, 9)
    diag = np.zeros((C, 58 * 32), np.float32)
    ar = np.arange(C)
    for t in range(49):
        diag[ar, t * 32 + (ar % 32)] = w7[:, t]
    for t in range(9):
        diag[ar, (49 + t) * 32 + (ar % 32)] = w3[:, t]
    out["wdiag"] = diag.astype(bfnp)

    # GEMM lhsT weights; bias-row tensors padded to 128 partitions (a
    # 97-partition DMA corrupts later bf16 loads).
    wfc1 = np.zeros((128, 384), np.float32)
    wfc1[:C] = fc1w.T
    wfc1[C] = fc1b
    out["wfc1"] = wfc1.astype(bfnp)
    wfc2 = np.zeros((128, 3 * C), np.float32)
    for j in range(3):
        wfc2[:, j * C:(j + 1) * C] = fc2w[:, j * 128:(j + 1) * 128].T
    out["wfc2"] = wfc2.astype(bfnp)
    wip = np.zeros((128, C), np.float32)
    wip[:C] = ipw.T
    wip[C] = ipb
    out["wip"] = wip.astype(bfnp)
    out["wop"] = opw.T.astype(bfnp)
    wm1 = np.zeros((128, 384), np.float32)
    wm1[:C] = mfc1w.T
    wm1[C] = mfc1b
    out["wmfc1"] = wm1.astype(bfnp)
    wm2 = np.zeros((128, 3 * C), np.float32)
    for j in range(3):
        wm2[:, j * C:(j + 1) * C] = mfc2w[:, j * 128:(j + 1) * 128].T
    out["wmfc2"] = wm2.astype(bfnp)

    # per-direction projections, [96,96] lhsT each
    xp = f(kw["x_proj_w"])
    dtw = f(kw["dt_w"])
    wz = np.zeros((C, 4 * C), np.float32)
    wnB = np.zeros((C, 4 * C), np.float32)
    wC = np.zeros((C, 4 * C), np.float32)
    for k in range(4):
        m = dtw[k] @ xp[k][:R]
        wz[:, k * C:(k + 1) * C] = -m.T
        wnB[:, k * C:(k + 1) * C] = -xp[k][R][:, None]
        wC[:, k * C:(k + 1) * C] = xp[k][R + 1][:, None]
    out["wz"] = wz.astype(bfnp)
    out["wnB"] = wnB.astype(bfnp)
    out["wC"] = wC.astype(bfnp)

    out["onesn_bf"] = np.full((C, C), -1.0 / C, bfnp)
    out["ones_f32"] = np.full((C, C), 1.0, np.float32)  # exact in bf16
    onesv = np.zeros((128, C), np.float32)
    onesv[:C + 1] = 1.0 / C
    out["onesv_bf"] = onesv.astype(bfnp)

    A = (-np.exp(f(kw["A_logs"]))).reshape(K, C)
    a_is_neg1 = bool(np.allclose(A, -1.0, atol=1e-6))
    Ds = f(kw["Ds"]).reshape(K, C)
    dtb = f(kw["dt_b"])
    cols = []

    def col(v):
        a = np.zeros(128, np.float32)
        a[: len(v)] = v
        cols.append(a)
        return len(cols) - 1

    ix = {}
    ix["cn_dw_b"] = col(f(kw["cn_dw_b"]))
    ix["fc2b"] = col(fc2b)
    ix["dwb"] = col(f(kw["dw_b"]))
    for k in range(4):
        ix[f"ndtb{k}"] = col(-dtb[k])
        ix[f"nA{k}"] = col(-A[k])
    ix["Dsum"] = col(Ds.sum(0))
    ix["opb"] = col(opb)
    ix["mfc2b"] = col(mfc2b)
    out["vecs"] = np.stack(cols, axis=1)
    return out, ix, a_is_neg1


def pad_image(x):
    xp = np.zeros((C, P7, P7), np.float32)
    xp[:, 3:3 + H, 3:3 + W] = x
    return xp.reshape(C, P7 * P7).astype(bfnp)


def r32(ap):
    return ap.bitcast(F32R)


def build_program(ix, a_is_neg1=True):
    nc = bacc.Bacc("TRN2", target_bir_lowering=False, debug=False)

    din = {}
    for name, shape, dt in [
        ("xpad", [C, P7 * P7], BF16),
        ("xres", [C, L], F32),
        ("wdiag", [C, 58 * 32], BF16),
        ("wfc1", [128, 384], BF16),
        ("wfc2", [128, 3 * C], BF16),
        ("wip", [128, C], BF16),
        ("wop", [C, C], BF16),
        ("wmfc1", [128, 384], BF16),
        ("wmfc2", [128, 3 * C], BF16),
        ("wz", [C, 4 * C], BF16),
        ("wnB", [C, 4 * C], BF16),
        ("wC", [C, 4 * C], BF16),
        ("onesn_bf", [C, C], BF16),
        ("ones_f32", [C, C], F32R),
        ("onesv_bf", [128, C], BF16),
        ("vecs", [128, len(ix)], F32),
    ]:
        din[name] = nc.dram_tensor(name, shape, dt, kind="ExternalInput").ap()
    dout = nc.dram_tensor("out", [C, L], F32, kind="ExternalOutput").ap()

    class ActPhase:
        def __init__(self):
            self.cur_last = None
            self.last_is_load = False

        def tag(self, bi, is_load=False):
            inst = bi.ins
            if self.cur_last is not None:
                add_dep_helper(inst, self.cur_last,
                               sync=not (is_load or self.last_is_load),
                               reason="act table-set phase fence")
            self.cur_last = inst
            self.last_is_load = is_load
            return bi

    ph = ActPhase()

    with tile.TileContext(nc) as tc:
        from contextlib import ExitStack

        with ExitStack() as ctx:
            const = ctx.enter_context(tc.tile_pool(name="const", bufs=1))
            bigp = ctx.enter_context(tc.tile_pool(name="big", bufs=1))
            lnp = ctx.enter_context(tc.tile_pool(name="ln", bufs=2))
            gp = ctx.enter_context(tc.tile_pool(name="g", bufs=2))
            sgp = ctx.enter_context(tc.tile_pool(name="sg", bufs=1))
            scanp = ctx.enter_context(tc.tile_pool(name="scan", bufs=3))
            hcp = ctx.enter_context(tc.tile_pool(name="hc", bufs=2))
            hp = ctx.enter_context(tc.tile_pool(name="hp", bufs=4))
            accp = ctx.enter_context(tc.tile_pool(name="acc", bufs=2))
            xrp = ctx.enter_context(tc.tile_pool(name="xr", bufs=1))
            ps = ctx.enter_context(tc.tile_pool(name="ps", bufs=1,
                                                space="PSUM"))

            # ---- PSUM: cv0-3 [128,512] rotation + st/fa [128,1024] ----
            cvi = [0]

            def cv_tile():
                t = ps.tile([128, CH], F32, tag=f"cv{cvi[0] % 4}",
                            name=f"cv{cvi[0]}")
                cvi[0] += 1
                return t

            sti = [0]

            def st_tile():
                t = ps.tile([128, PR], F32, tag="st", name=f"st{sti[0]}")
                sti[0] += 1
                return t

            fai = [0]

            def fa_tile():
                t = ps.tile([128, PR], F32, tag="fa", name=f"fa{fai[0]}")
                fai[0] += 1
                return t

            # ---- constant loads (xpad + wdiag + vecs first) ----
            cc = {}
            order = ["vecs", "wdiag", "onesn_bf", "onesv_bf", "wfc1",
                     "wfc2", "ones_f32", "wip", "wz", "wnB", "wC", "wop",
                     "wmfc1", "wmfc2"]
            xpad = bigp.tile([C, P7 * P7], BF16, tag="pad")
            nc.sync.dma_start(xpad[:], din["xpad"])
            for name in order:
                ap = din[name]
                t = const.tile(list(ap.shape), ap.dtype, tag=name, name=name)
                nc.sync.dma_start(t[:], ap)
                cc[name] = t

            def xres_pair(p):
                t = xrp.tile([C, PR], F32, tag="xr", name=f"xr{p}")
                nc.sync.dma_start(t[:], din["xres"][:, p * PR:(p + 1) * PR])
                return t

            nv = len(ix)
            vecs_sb = const.tile([128, nv], F32, tag="vecs_sb")
            _ld0 = mybir.InstLoadActFuncSet(
                name=nc.get_next_instruction_name(), ins=[], outs=[])
            _ld0.act_func_set_id = 6
            ph.tag(nc.scalar.add_instruction(_ld0), is_load=True)
            ph.tag(nc.scalar.activation(vecs_sb[:], cc["vecs"][:], AF.Copy))
            scr = const.tile([128, 1], F32, tag="scr")
            ph.tag(nc.scalar.activation(scr[:], vecs_sb[:, 0:1], AF.Copy))
            V96 = lambda key: vecs_sb[:C, ix[key]:ix[key] + 1]

            def load_set(set_id):
                ld = mybir.InstLoadActFuncSet(
                    name=nc.get_next_instruction_name(), ins=[], outs=[])
                ld.act_func_set_id = set_id
                return ph.tag(nc.scalar.add_instruction(ld), is_load=True)

            # ---- helpers ----
            def conv_group(src3, ktaps, diag_off, jlist):
                # tap-outer across a group of chunks: consecutive matmuls
                # hit different psum banks, so the PE overlaps them (the
                # within-bank accumulation chain alone would serialize).
                taps, _ = _taps(ktaps)
                nt = len(taps)
                pts = [cv_tile() for _ in jlist]
                for t, (dh, dw) in enumerate(taps):
                    wcol = cc["wdiag"][:, (diag_off + t) * 32:
                                       (diag_off + t + 1) * 32]
                    for i, j in enumerate(jlist):
                        r0 = j * 8
                        rhs = src3[:, r0 + dh:r0 + dh + 8, dw:dw + W]
                        for g in range(3):
                            nc.tensor.matmul(
                                pts[i][32 * g:32 * (g + 1), :],
                                wcol[32 * g:32 * (g + 1), :],
                                rhs[32 * g:32 * (g + 1)],
                                start=(t == 0), stop=(t == nt - 1),
                                skip_group_check=True,
                                tile_position=(32 * g, 32 * g),
                            )
                return pts

            dsq_tiles = []
            for i in range(2):
                t = lnp.tile([C + 1, PR], BF16, tag="dsq", name=f"dsq{i}")
                nc.gpsimd.memset(t[C:C + 1, :], float(EPS * C))
                dsq_tiles.append(t)

            # LN split into stages so the caller can interleave emission:
            # ln_front = mean + d + sq; ln_back = var + Ln + Exp + xn.
            def ln_front(src_tile, src_f32, p, ptile=None):
                mb = (ptile or st_tile)()
                ones = cc["ones_f32"] if src_f32 else cc["onesn_bf"]
                for h in range(2):
                    rhs = src_tile[:, p * PR + h * CH: p * PR + (h + 1) * CH]
                    nc.tensor.matmul(mb[0:C, h * CH:(h + 1) * CH], ones[:],
                                     r32(rhs) if src_f32 else rhs,
                                     start=True, stop=True)
                src = src_tile[:, p * PR:(p + 1) * PR]
                dloc = lnp.tile([C, PR], BF16, tag="d", name=f"d{p}")
                if src_f32:
                    # mb holds +sum (exact-1.0 f32r ones): d = -sum/96 + x
                    nc.vector.scalar_tensor_tensor(dloc[:], mb[0:C, :],
                                                   -1.0 / C, src,
                                                   OP.mult, OP.add)
                else:
                    nc.vector.tensor_tensor(dloc[:], src, mb[0:C, :], OP.add)
                dsq = dsq_tiles[p % 2]
                nc.gpsimd.tensor_tensor(dsq[0:C, :], dloc[:], dloc[:],
                                        OP.mult)
                return dloc

            def ln_back(dloc, xn, p, ptile=None):
                dsq = dsq_tiles[p % 2]
                vb = (ptile or st_tile)()
                for h in range(2):
                    nc.tensor.matmul(vb[0:C, h * CH:(h + 1) * CH],
                                     cc["onesv_bf"][0:C + 1, :],
                                     dsq[:, h * CH:(h + 1) * CH],
                                     start=True, stop=True)
                lnv = lnp.tile([C, PR], BF16, tag="lnv", name=f"lnv{p}")
                ph.tag(nc.scalar.activation(lnv[:], vb[0:C, :], AF.Ln))
                rstd = lnp.tile([C, PR], BF16, tag="rstd", name=f"rs{p}")
                ph.tag(nc.scalar.activation(rstd[:], lnv[:], AF.Exp,
                                            scale=-0.5))
                nc.vector.tensor_tensor(xn[0:C, p * PR:(p + 1) * PR],
                                        dloc[:], rstd[:], OP.mult)

            # =============== Phase A: conv7 + ConvNeXt ===============
            xpad3 = xpad[:].rearrange("c (h w) -> c h w", w=P7)
            hsb = bigp.tile([C, L], BF16, tag="hsb")
            xn = bigp.tile([C + 1, L], BF16, tag="xn")
            nc.gpsimd.memset(xn[C:C + 1, :], 1.0)

            # conv in two 4-chunk tap-outer groups; group B's PE work
            # covers group A's LN chains.
            ptsA = conv_group(xpad3, 7, 0, [0, 1, 2, 3])
            for j in range(4):
                ph.tag(nc.scalar.activation(
                    hsb[:, j * CH:(j + 1) * CH], ptsA[j][0:C, :],
                    AF.Identity, bias=V96("cn_dw_b")))
            d0 = ln_front(hsb, False, 0)
            d1 = ln_front(hsb, False, 1)
            ptsB = conv_group(xpad3, 7, 0, [4, 5, 6, 7])
            ln_back(d0, xn, 0)
            ln_back(d1, xn, 1)
            for j in range(4, 8):
                ph.tag(nc.scalar.activation(
                    hsb[:, j * CH:(j + 1) * CH], ptsB[j - 4][0:C, :],
                    AF.Identity, bias=V96("cn_dw_b")))
            d2 = ln_front(hsb, False, 2)
            d3 = ln_front(hsb, False, 3)
            ln_back(d2, xn, 2)
            ln_back(d3, xn, 3)

            # fc sweep: fc1 chunks through cv rotation, fc2 accum in fa
            x1 = bigp.tile([C, L], BF16, tag="x1")
            load_set(10)
            for p in range(NPR):
                xrc = xres_pair(p)
                f2 = fa_tile()
                for m in range(3):
                    wsl = cc["wfc1"][0:C + 1, m * 128:(m + 1) * 128]
                    g = gp.tile([128, PR], BF16, tag="g", name=f"g{p}_{m}")
                    for h in range(2):
                        f1 = cv_tile()
                        nc.tensor.matmul(
                            f1[:, :], wsl,
                            xn[:, p * PR + h * CH: p * PR + (h + 1) * CH],
                            start=True, stop=True)
                        ph.tag(nc.scalar.activation(
                            g[:, h * CH:(h + 1) * CH], f1[:], AF.Gelu))
                    for h in range(2):
                        nc.tensor.matmul(f2[0:C, h * CH:(h + 1) * CH],
                                         cc["wfc2"][:, m * C:(m + 1) * C],
                                         g[:, h * CH:(h + 1) * CH],
                                         start=(m == 0), stop=(m == 2))
                nc.vector.scalar_tensor_tensor(
                    x1[:, p * PR:(p + 1) * PR], f2[0:C, :],
                    V96("fc2b"), xrc[:], OP.add, OP.add)

            # =============== Phase B: LN1 + ip + conv3 + silu ============
            xn1 = bigp.tile([C + 1, L], BF16, tag="xn1")
            nc.gpsimd.memset(xn1[C:C + 1, :], 1.0)
            load_set(6)
            dl = None
            for p in range(NPR):
                d_now = ln_front(x1, False, p,
                                 st_tile if p % 2 == 0 else fa_tile)
                if dl is not None:
                    ln_back(dl, xn1, p - 1,
                            st_tile if (p - 1) % 2 == 0 else fa_tile)
                dl = d_now
            ln_back(dl, xn1, 3, fa_tile)

            v2pad_full = bigp.tile([C, P7 * P7], BF16, tag="pad")
            v2pad = v2pad_full[:, 0:P3 * P3]
            nc.gpsimd.memset(v2pad, 0.0)
            v2int = v2pad.rearrange("c (h w) -> c h w", w=P3)
            for j in range(NCH):
                pv = cv_tile()
                nc.tensor.matmul(pv[0:C, :], cc["wip"][0:C + 1, :],
                                 xn1[:, j * CH:(j + 1) * CH],
                                 start=True, stop=True)
                dst = v2int[:, 1 + j * 8:1 + (j + 1) * 8, 1:1 + W]
                nc.vector.tensor_scalar(dst, pv[0:C, :], 0.0, None, OP.add)

            v4 = bigp.tile([C, L], BF16, tag="hsb")
            load_set(18)
            c3a = conv_group(v2int, 3, 49, [0, 1, 2, 3])
            for j in range(4):
                ph.tag(nc.scalar.activation(v4[:, j * CH:(j + 1) * CH],
                                            c3a[j][0:C, :], AF.Silu,
                                            bias=V96("dwb")))
            c3b = conv_group(v2int, 3, 49, [4, 5, 6, 7])
            for j in range(4, 8):
                ph.tag(nc.scalar.activation(v4[:, j * CH:(j + 1) * CH],
                                            c3b[j - 4][0:C, :], AF.Silu,
                                            bias=V96("dwb")))

            # =============== Phase C: 4-direction scan ===============
            v4T = v4[:].rearrange("c (h w) -> c h w", w=W).transpose(
                [0, 2, 1])

            def urhs_full(k):
                return v4[:] if k in (0, 2) else v4T

            def useg(k, px0, npx):
                if k in (0, 2):
                    return v4[:, px0:px0 + npx]
                return v4T[:, px0 // H:(px0 + npx) // H, :]

            def wcol(name, k):
                return cc[name][:, k * C:(k + 1) * C]

            sg_tiles = {}
            hs = {}
            accs = {}

            def z_pass(k):
                sg = sgp.tile([C, L], BF16, tag=f"sg{k}", name=f"sg{k}")
                sg_tiles[k] = sg
                for j in range(NCH):
                    zp = cv_tile()
                    nc.tensor.matmul(zp[0:C, :], wcol("wz", k),
                                     useg(k, j * CH, CH), start=True,
                                     stop=True)
                    ph.tag(nc.scalar.activation(
                        sg[:, j * CH:(j + 1) * CH], zp[0:C, :], AF.Sigmoid,
                        bias=V96(f"ndtb{k}")))

            def rank1_pass(name, k, dst, eng=("act", "dve")):
                for j in range(NCH):
                    bb = cv_tile()
                    s = slice(j * CH, (j + 1) * CH)
                    nc.tensor.matmul(bb[0:C, :], wcol(name, k),
                                     useg(k, j * CH, CH), start=True,
                                     stop=True)
                    if eng[j % 2] == "act":
                        ph.tag(nc.scalar.activation(dst[:, s], bb[0:C, :],
                                                    AF.Copy))
                    else:
                        nc.vector.tensor_scalar(dst[:, s], bb[0:C, :], 0.0,
                                                None, OP.add)

            def scan_dir(k):
                lnd = scanp.tile([C, L], BF16, tag="sc", name=f"lnd{k}")
                ph.tag(nc.scalar.activation(lnd[:], sg_tiles[k][:], AF.Ln))
                if a_is_neg1:
                    dA = sg_tiles[k]
                else:
                    dA = sgp.tile([C, L], BF16, tag=f"dA{k >> 1}",
                                  name=f"dA{k}")
                    ph.tag(nc.scalar.activation(dA[:], lnd[:], AF.Exp,
                                                scale=V96(f"nA{k}")))
                du = scanp.tile([C, L], BF16, tag="sc", name=f"du{k}")
                nc.vector.tensor_tensor(du[:], lnd[:], urhs_full(k), OP.mult)
                bbe = scanp.tile([C, L], BF16, tag="sc", name=f"bbe{k}")
                rank1_pass("wnB", k, bbe)
                bso = scanp.tile([C, L], BF16, tag="sc", name=f"bso{k}")
                nc.vector.tensor_tensor(bso[:], du[:], bbe[:], OP.mult)
                h = hp.tile([C, L], BF16, tag="h", name=f"h{k}")
                if k in (0, 1):
                    nc.vector.tensor_tensor_scan(h[:], dA[:], bso[:],
                                                 0.0, OP.mult, OP.add)
                else:
                    nc.vector.tensor_tensor_scan(h[:][:, ::-1],
                                                 dA[:][:, ::-1],
                                                 bso[:][:, ::-1], 0.0,
                                                 OP.mult, OP.add)
                return h

            ORDER = (0, 2, 1, 3)
            for k in ORDER:
                load_set(2)
                z_pass(k)
                load_set(6)
                hs[k] = scan_dir(k)
            for k in ORDER:
                cbe = scanp.tile([C, L], BF16, tag="sc", name=f"cbe{k}")
                rank1_pass("wC", k, cbe)
                if k in (0, 1):
                    dst = hcp.tile([C, L], BF16, tag="hc", name=f"hc{k}")
                    nc.vector.tensor_tensor(dst[:], hs[k][:], cbe[:],
                                            OP.mult)
                else:
                    tmp = scanp.tile([C, L], BF16, tag="sc", name=f"y{k}")
                    if k == 2:
                        nc.vector.tensor_tensor(tmp[:], hs[k][:],
                                                cbe[:], OP.mult)
                        dst = accp.tile([C, L], BF16, tag="acc",
                                        name=f"acc{k}")
                        nc.gpsimd.tensor_tensor(dst[:], accs[k - 2][:],
                                                tmp[:], OP.add)
                    else:
                        nc.gpsimd.tensor_tensor(tmp[:], hs[k][:],
                                                cbe[:], OP.mult)
                        dst = accp.tile([C, L], BF16, tag="acc",
                                        name=f"acc{k}")
                        nc.vector.tensor_tensor(dst[:], accs[k - 2][:],
                                                tmp[:], OP.add)
                accs[k] = dst

            # =============== merge + LN + op + LN2 + MLP2 ==============
            preln = hcp.tile([C, L], BF16, tag="hc")
            accT = accs[3][:].rearrange("c (w h) -> c w h", w=W).transpose(
                [0, 2, 1])
            t2 = scanp.tile([C, L], BF16, tag="sc", name="t2")
            nc.vector.tensor_tensor(t2[:], accs[2][:], accT, OP.add)
            nc.vector.scalar_tensor_tensor(preln[:], v4[:], V96("Dsum"),
                                           t2[:], OP.mult, OP.add)

            ynf = bigp.tile([C + 1, L], BF16, tag="xn")  # xn dead
            x2 = bigp.tile([C, L], BF16, tag="x2")
            def op_back(dloc, p):
                ln_back(dloc, ynf, p)
                po = fa_tile()
                for h in range(2):
                    s = slice(p * PR + h * CH, p * PR + (h + 1) * CH)
                    nc.tensor.matmul(po[0:C, h * CH:(h + 1) * CH],
                                     cc["wop"][:], ynf[0:C, s], start=True,
                                     stop=True)
                nc.vector.scalar_tensor_tensor(
                    x2[:, p * PR:(p + 1) * PR], po[0:C, :],
                    V96("opb"), x1[:, p * PR:(p + 1) * PR], OP.add, OP.add)

            dprev = None
            for p in range(NPR):
                d_now = ln_front(preln, False, p)
                if dprev is not None:
                    op_back(dprev, p - 1)
                dprev = d_now
            op_back(dprev, 3)

            mxn = bigp.tile([C + 1, L], BF16, tag="xn1")
            nc.gpsimd.memset(mxn[C:C + 1, :], 1.0)
            dl = None
            for p in range(NPR):
                d_now = ln_front(x2, False, p,
                                 st_tile if p % 2 == 0 else fa_tile)
                if dl is not None:
                    ln_back(dl, mxn, p - 1,
                            st_tile if (p - 1) % 2 == 0 else fa_tile)
                dl = d_now
            ln_back(dl, mxn, 3, fa_tile)

            outp = ctx.enter_context(tc.tile_pool(name="outp", bufs=1))
            load_set(10)
            for p in range(NPR):
                f2 = fa_tile()
                for m in range(3):
                    wsl = cc["wmfc1"][0:C + 1, m * 128:(m + 1) * 128]
                    g = gp.tile([128, PR], BF16, tag="g", name=f"mg{p}_{m}")
                    for h in range(2):
                        f1 = cv_tile()
                        nc.tensor.matmul(
                            f1[:, :], wsl,
                            mxn[:, p * PR + h * CH: p * PR + (h + 1) * CH],
                            start=True, stop=True)
                        ph.tag(nc.scalar.activation(
                            g[:, h * CH:(h + 1) * CH], f1[:], AF.Gelu))
                    for h in range(2):
                        nc.tensor.matmul(f2[0:C, h * CH:(h + 1) * CH],
                                         cc["wmfc2"][:, m * C:(m + 1) * C],
                                         g[:, h * CH:(h + 1) * CH],
                                         start=(m == 0), stop=(m == 2))
                sl = slice(p * PR, (p + 1) * PR)
                ob = outp.tile([C, PR], F32, tag="ob", name=f"ob{p}")
                nc.vector.scalar_tensor_tensor(ob[:], f2[0:C, :],
                                               V96("mfc2b"), x2[:, sl],
                                               OP.add, OP.add)
                nc.sync.dma_start(dout[:, sl], ob[:])

    nc.compile()
    return nc


def get_program_and_inputs(inputs):
    host, ix, a_is_neg1 = build_host_tensors(inputs)
    key = ("prog", a_is_neg1)
    if key not in _CACHE:
        _CACHE[key] = build_program(ix, a_is_neg1)
    nc = _CACHE[key]
    x = np.asarray(inputs["x"], np.float32)
    in_maps = []
    for b in range(B):
        m = {k: v for k, v in host.items()}
        m["xpad"] = pad_image(x[b])
        m["xres"] = x[b].reshape(C, L).astype(np.float32)
        in_maps.append(m)
    return nc, in_maps


def kernel(**inputs):
    nc, in_maps = get_program_and_inputs(inputs)
    res = run_bass_kernel_spmd(nc, in_maps, list(range(B)))
    out = np.stack([res.results[b]["out"].reshape(C, H, W) for b in range(B)])
    return out.astype(np.float32)


if __name__ == "__main__":
    host, ix, a1 = build_host_tensors(
        {k: np.zeros(s, np.float32) for k, s in [  # noqa
            ("x", (B, C, H, W)), ("cn_dw_w", (C, 7, 7)), ("cn_dw_b", (C,)),
            ("cn_ln_w", (C,)), ("cn_ln_b", (C,)), ("cn_fc1_w", (4 * C, C)),
            ("cn_fc1_b", (4 * C,)), ("cn_fc2_w", (C, 4 * C)),
            ("cn_fc2_b", (C,)), ("v_ln1_w", (C,)), ("v_ln1_b", (C,)),
            ("ip_w", (C, C)), ("ip_b", (C,)), ("dw_w", (C, 3, 3)),
            ("dw_b", (C,)), ("x_proj_w", (K, R + 2 * N, C)),
            ("dt_w", (K, C, R)), ("dt_b", (K, C)), ("A_logs", (K * C, N)),
            ("Ds", (K * C,)), ("o_ln_w", (C,)), ("o_ln_b", (C,)),
            ("op_w", (C, C)), ("op_b", (C,)), ("v_ln2_w", (C,)),
            ("v_ln2_b", (C,)), ("m_fc1_w", (4 * C, C)),
            ("m_fc1_b", (4 * C,)), ("m_fc2_w", (C, 4 * C)),
            ("m_fc2_b", (C,)),
        ]})
    print("a_is_neg1:", a1)
    nc = build_program(ix, a1)
    print("program built OK:", len(list(nc.all_instructions())),
          "instructions")
